# revision 7
# baseline (speedup 1.0000x reference)
"""AUGRU Trainium2 kernel v4 — shortened recurrence chain.

Layout (unchanged from v3): b = 8j+k strided across cores; j = 2c+half
interleaved halves stacked on partitions; on-chip tensors [128, cols<=256].

v4 chain shaves vs v3:
  - Block-diagonal weights [128,128] (diag(W.T, W.T)): ONE matmul per gate
    per step instead of two per-half matmuls; no N2 bookkeeping (the dead
    half-B columns compute bounded garbage that is masked on the host and
    never revived).
  - h' = uu - vv is REMOVED from the critical chain: the next step's
    h-side matmuls are distributed over the two addends,
    W.h = W.uu + (-W).vv, using negated weight copies for the vv side.
    vv = (zw-1) (.) h is ready mid-step (z-path), so its matmuls are queued
    ahead of the uu-side r-gate matmul, which then pipelines right behind.
    The chain is: mm_uu_r -> sigmoid(r) -> m1 -> a1 -> tanh -> uu -> (next).
  - psum gate outputs evacuated to bf16 SBUF off the chain (nib = copy of
    pni; nhb = pnh + bhn), so on-chain m1/a1 are 2x-mode bf16 DVE ops.
  - z-path off the vector queue: zw = zs (.) w and vv on GpSimd.
  - h' (sub) runs off-chain on the vector queue, feeding only vv(t+1) and
    the output DMA.
"""

import os
import ml_dtypes
import numpy as np

import concourse.bass as bass
import concourse.bacc as bacc
import concourse.mybir as mybir
from concourse.tile import TileContext
from concourse.bass_utils import run_bass_kernel_spmd

T, B, D, H = 200, 4096, 64, 64
NCORES = 8
BC = B // NCORES  # 512 batch rows per core
HALF = BC // 2    # 256 columns per half
P = T // 2        # step pairs

LAST_RESULT = None

f32 = mybir.dt.float32
bf16 = mybir.dt.bfloat16
AF = mybir.ActivationFunctionType
ALU = mybir.AluOpType

WNAMES = ["xr", "xz", "xn", "ur", "uz", "un", "vr", "vz", "vn"]


def _build_program(N1s):
    nc = bacc.Bacc()

    x_d = nc.declare_dram_parameter("x", [P, 128, 2, HALF], bf16, isOutput=False)
    w_d = nc.declare_dram_parameter("w", [P, 128, 2, HALF], bf16, isOutput=False)
    wd = {
        name: nc.declare_dram_parameter(name, [128, 128], bf16, isOutput=False)
        for name in WNAMES
    }
    br_d = nc.declare_dram_parameter("br", [128, 1], f32, isOutput=False)
    bz_d = nc.declare_dram_parameter("bz", [128, 1], f32, isOutput=False)
    bhn_d = nc.declare_dram_parameter("bhn", [128, 1], f32, isOutput=False)
    bin_d = nc.declare_dram_parameter("bin", [128, 1], f32, isOutput=False)
    op_d = nc.declare_dram_parameter("op", [P, 128, 2, HALF], bf16, isOutput=True)

    with TileContext(nc) as tc:
        with (
            tc.tile_pool(name="const", bufs=1) as cpool,
            tc.tile_pool(name="hb", bufs=3) as hbpool,
            tc.tile_pool(name="xin", bufs=3) as xpool,
            tc.tile_pool(name="win", bufs=3) as wpool,
            tc.tile_pool(name="work", bufs=2) as spool,
            tc.tile_pool(name="ps", bufs=2, space="PSUM") as ppool,
        ):
            wts = {}
            for name in WNAMES:
                t_ = cpool.tile([128, 128], bf16, tag=name)
                nc.sync.dma_start(out=t_[:, :], in_=wd[name][:, :])
                wts[name] = t_
            biases = {}
            for name, dram in [("br", br_d), ("bz", bz_d), ("bhn", bhn_d),
                               ("bin", bin_d)]:
                t_ = cpool.tile([128, 1], f32, tag=name)
                nc.sync.dma_start(out=t_[:, :], in_=dram[:, :])
                biases[name] = t_

            wv0 = int(N1s[0])
            x_cur = xpool.tile([128, 2, HALF], bf16, tag="x")
            nc.sync.dma_start(out=x_cur[:, :, 0:wv0], in_=x_d[0, :, :, 0:wv0])
            pw_cur = wpool.tile([128, 2, HALF], bf16, tag="pw")
            nc.sync.dma_start(out=pw_cur[:, :, 0:wv0], in_=w_d[0, :, :, 0:wv0])
            x_nxt = pw_nxt = None

            # psum for step 0: x-side only (h_{-1} = 0)
            pr_c = ppool.tile([128, HALF], f32, tag="pr")
            pz_c = ppool.tile([128, HALF], f32, tag="pz")
            pn_c = ppool.tile([128, 2 * HALF], f32, tag="pn")
            nc.tensor.matmul(pr_c[:, 0:wv0], lhsT=wts["xr"][:, :],
                             rhs=x_cur[:, 0, 0:wv0], start=True, stop=True)
            nc.tensor.matmul(pz_c[:, 0:wv0], lhsT=wts["xz"][:, :],
                             rhs=x_cur[:, 0, 0:wv0], start=True, stop=True)
            nc.tensor.matmul(pn_c[:, 0:wv0], lhsT=wts["xn"][:, :],
                             rhs=x_cur[:, 0, 0:wv0], start=True, stop=True)

            nib_c = nhb_c = None
            hpair = None
            h_prev = None  # AP of h_{t-1} slice [128, HALF]

            for t in range(T):
                N = int(N1s[t])
                if N == 0:
                    if t % 2 == 1 and hpair is not None:
                        wide = int(N1s[t - 1])
                        nc.sync.dma_start(out=op_d[t // 2, :, 0:1, 0:wide],
                                          in_=hpair[:, 0:1, 0:wide])
                    break
                p, s = divmod(t, 2)
                Nn = int(N1s[t + 1]) if t + 1 < T else 0

                if s == 0:
                    hpair = hbpool.tile([128, 2, HALF], bf16, tag="hb")
                    x_nxt = xpool.tile([128, 2, HALF], bf16, tag="x")
                    pw_nxt = wpool.tile([128, 2, HALF], bf16, tag="pw")
                    if p + 1 < P:
                        wvn = int(N1s[2 * (p + 1)])
                        if wvn > 0:
                            nc.sync.dma_start(out=x_nxt[:, :, 0:wvn],
                                              in_=x_d[p + 1, :, :, 0:wvn])
                            nc.sync.dma_start(out=pw_nxt[:, :, 0:wvn],
                                              in_=w_d[p + 1, :, :, 0:wvn])

                pni_c = pn_c[:, 0:HALF]
                pnh_c = pn_c[:, HALF:2 * HALF]

                # ---- ACT: sigmoid(r) leads the chain; sigmoid(z) follows
                rs = spool.tile([128, HALF], bf16, tag="rs")
                nc.scalar.activation(rs[:, 0:N], pr_c[:, 0:N], AF.Sigmoid,
                                     bias=biases["br"][:, 0:1], scale=1.0)
                zs = spool.tile([128, HALF], bf16, tag="zs")
                nc.scalar.activation(zs[:, 0:N], pz_c[:, 0:N], AF.Sigmoid,
                                     bias=biases["bz"][:, 0:1], scale=1.0)

                # ---- PE: x-side matmuls for step t+1 (run early, no deps)
                if Nn > 0:
                    pr_n = ppool.tile([128, HALF], f32, tag="pr")
                    pz_n = ppool.tile([128, HALF], f32, tag="pz")
                    pn_n = ppool.tile([128, 2 * HALF], f32, tag="pn")
                    xs = 1 - s
                    xt = x_cur if s == 0 else x_nxt
                    nc.tensor.matmul(pr_n[:, 0:Nn], lhsT=wts["xr"][:, :],
                                     rhs=xt[:, xs, 0:Nn], start=True, stop=False)
                    nc.tensor.matmul(pz_n[:, 0:Nn], lhsT=wts["xz"][:, :],
                                     rhs=xt[:, xs, 0:Nn], start=True, stop=False)
                    nc.tensor.matmul(pn_n[:, 0:Nn], lhsT=wts["xn"][:, :],
                                     rhs=xt[:, xs, 0:Nn], start=True, stop=True)
                else:
                    pr_n = pz_n = pn_n = None

                # ---- chain: m1 = rs (.) (pnh + bhn); a1 = m1 + pni
                m1 = spool.tile([128, HALF], bf16, tag="m1")
                if t == 0:
                    nc.vector.tensor_scalar_mul(m1[:, 0:N], rs[:, 0:N],
                                                biases["bhn"][:, 0:1])
                else:
                    nc.vector.tensor_mul(m1[:, 0:N], rs[:, 0:N], nhb_c[:, 0:N])
                a1 = spool.tile([128, HALF], bf16, tag="a1")
                if t == 0:
                    nib_c = spool.tile([128, HALF], bf16, tag="nib")
                    nc.vector.tensor_scalar_add(nib_c[:, 0:N], pni_c[:, 0:N],
                                                0.0)
                nc.vector.tensor_add(a1[:, 0:N], m1[:, 0:N], nib_c[:, 0:N])

                nt = spool.tile([128, HALF], bf16, tag="nt")
                nc.scalar.activation(nt[:, 0:N], a1[:, 0:N], AF.Tanh,
                                     bias=biases["bin"][:, 0:1], scale=1.0)

                # ---- z path: zw = zs (.) w ; vv = (zw-1) (.) h
                zw = spool.tile([128, HALF], bf16, tag="zw")
                nc.vector.tensor_mul(zw[:, 0:N], zs[:, 0:N], pw_cur[:, s, 0:N])
                if t > 0:
                    vv = spool.tile([128, HALF], bf16, tag="vv")
                    nc.vector.scalar_tensor_tensor(
                        out=vv[:, 0:N], in0=zw[:, 0:N], scalar=1.0,
                        in1=h_prev[:, 0:N], op0=ALU.subtract, op1=ALU.mult)

                # ---- chain tail: uu = zw (.) nt
                uu = spool.tile([128, HALF], bf16, tag="uu")
                nc.vector.tensor_mul(uu[:, 0:N], zw[:, 0:N], nt[:, 0:N])

                # ---- PE: h-side matmuls for step t+1 on (uu, vv)
                # order: vv_r, uu_r(stop) first so sigma_r(t+1) fires early;
                # then the n-bank (for nhb), then the z-bank.
                if Nn > 0:
                    pni_n = pn_n[:, 0:HALF]
                    pnh_n = pn_n[:, HALF:2 * HALF]
                    if t > 0:
                        nc.tensor.matmul(pr_n[:, 0:Nn], lhsT=wts["vr"][:, :],
                                         rhs=vv[:, 0:Nn], start=False,
                                         stop=False)
                    nc.tensor.matmul(pr_n[:, 0:Nn], lhsT=wts["ur"][:, :],
                                     rhs=uu[:, 0:Nn], start=False, stop=True)
                    if t > 0:
                        nc.tensor.matmul(pnh_n[:, 0:Nn], lhsT=wts["vn"][:, :],
                                         rhs=vv[:, 0:Nn], start=True,
                                         stop=False)
                    nc.tensor.matmul(pnh_n[:, 0:Nn], lhsT=wts["un"][:, :],
                                     rhs=uu[:, 0:Nn], start=(t == 0),
                                     stop=True)
                    if t > 0:
                        nc.tensor.matmul(pz_n[:, 0:Nn], lhsT=wts["vz"][:, :],
                                         rhs=vv[:, 0:Nn], start=False,
                                         stop=False)
                    nc.tensor.matmul(pz_n[:, 0:Nn], lhsT=wts["uz"][:, :],
                                     rhs=uu[:, 0:Nn], start=False, stop=True)

                    # nib/nhb(t+1): evacuate pni/pnh(t+1) to bf16, after uu
                    # so they never block the chain ops on the vector queue
                    nib_n = spool.tile([128, HALF], bf16, tag="nib")
                    nc.vector.tensor_scalar_add(nib_n[:, 0:Nn],
                                                pn_n[:, 0:Nn], 0.0)
                    nhb_n = spool.tile([128, HALF], bf16, tag="nhb")
                    nc.vector.tensor_scalar_add(nhb_n[:, 0:Nn],
                                                pnh_n[:, 0:Nn],
                                                biases["bhn"][:, 0:1])
                else:
                    nib_n = nhb_n = None

                # ---- off-chain: h_t = uu - vv (for vv(t+1) + output);
                # on GpSimd to keep the vector queue light
                if t == 0:
                    nc.vector.tensor_scalar_add(hpair[:, s, 0:N], uu[:, 0:N],
                                                0.0)
                else:
                    nc.gpsimd.tensor_sub(hpair[:, s, 0:N], uu[:, 0:N],
                                         vv[:, 0:N])
                h_prev = hpair[:, s, :]

                if s == 1:
                    wide = int(N1s[2 * p])
                    nc.sync.dma_start(out=op_d[p, :, 0:2, 0:wide],
                                      in_=hpair[:, 0:2, 0:wide])
                    x_cur, pw_cur = x_nxt, pw_nxt

                pr_c, pz_c, pn_c = pr_n, pz_n, pn_n
                nib_c, nhb_c = nib_n, nhb_n

    nc.compile()
    return nc


def kernel(x, weights, lengths, W_ih, W_hh, b_ih, b_hh):
    global LAST_RESULT
    x = np.asarray(x, dtype=np.float32)
    weights = np.asarray(weights, dtype=np.float32)
    lengths = np.asarray(lengths, dtype=np.int32)
    W_ih = np.asarray(W_ih, dtype=np.float32)
    W_hh = np.asarray(W_hh, dtype=np.float32)
    b_ih = np.asarray(b_ih, dtype=np.float32)
    b_hh = np.asarray(b_hh, dtype=np.float32)

    counts = (lengths[None, :] > np.arange(T)[:, None]).sum(axis=1)  # [T]
    v = -(-counts // NCORES)
    N1s = (v + 1) // 2

    # x: [T,B,D] -> per-core [P, 128, 2, HALF] (feat-major, halves stacked,
    # two steps per block)
    xr = x.reshape(T, BC, NCORES, D)            # [t, j, k, d], b = 8j+k
    xr = xr.transpose(2, 0, 3, 1)               # [k, t, d, j]
    xr = xr.reshape(NCORES, T, D, HALF, 2)      # j = 2c + half
    xr = xr.transpose(0, 1, 4, 2, 3)            # [k, t, half, d, c]
    x8 = xr.reshape(NCORES, T, 128, HALF).astype(ml_dtypes.bfloat16)
    x8 = np.ascontiguousarray(
        x8.reshape(NCORES, P, 2, 128, HALF).transpose(0, 1, 3, 2, 4))

    wr = weights[:, :, 0].reshape(T, BC, NCORES)   # [t, j, k]
    wr = wr.transpose(2, 0, 1)                     # [k, t, j]
    wr = wr.reshape(NCORES, T, HALF, 2).transpose(0, 1, 3, 2)  # [k,t,half,c]
    w8 = np.broadcast_to(wr[:, :, :, None, :],
                         (NCORES, T, 2, 64, HALF))  # [k, t, half, d, c]
    w8 = w8.reshape(NCORES, T, 128, HALF)
    w8 = np.ascontiguousarray(
        w8.reshape(NCORES, P, 2, 128, HALF).transpose(0, 1, 3, 2, 4)
    ).astype(ml_dtypes.bfloat16)

    def blockdiag(wg):
        bd = np.zeros((128, 128), dtype=np.float32)
        bd[0:64, 0:64] = wg.T
        bd[64:128, 64:128] = wg.T
        return bd.astype(ml_dtypes.bfloat16)

    wts = {
        "xr": blockdiag(W_ih[0:64]),
        "xz": blockdiag(W_ih[64:128]),
        "xn": blockdiag(W_ih[128:192]),
        "ur": blockdiag(W_hh[0:64]),
        "uz": blockdiag(W_hh[64:128]),
        "un": blockdiag(W_hh[128:192]),
        "vr": blockdiag(-W_hh[0:64]),
        "vz": blockdiag(-W_hh[64:128]),
        "vn": blockdiag(-W_hh[128:192]),
    }
    b_r = (b_ih[0:64] + b_hh[0:64]).astype(np.float32)
    b_z = (b_ih[64:128] + b_hh[64:128]).astype(np.float32)
    b_hn = b_hh[128:192].astype(np.float32)
    b_in = b_ih[128:192].astype(np.float32)
    biases = {
        "br": np.tile(b_r, 2).reshape(128, 1),
        "bz": np.tile(b_z, 2).reshape(128, 1),
        "bhn": np.tile(b_hn, 2).reshape(128, 1),
        "bin": np.tile(b_in, 2).reshape(128, 1),
    }
    nc = _build_program(N1s)

    in_maps = []
    for k in range(NCORES):
        m = {"x": x8[k], "w": w8[k]}
        m.update(wts)
        m.update(biases)
        in_maps.append(m)

    trace = bool(os.environ.get("AUGRU_TRACE"))
    tmpdir = os.environ.get("AUGRU_PROF_DIR") or None
    res = run_bass_kernel_spmd(nc, in_maps, list(range(NCORES)), trace=trace,
                               tmpdir=tmpdir)
    LAST_RESULT = res

    # op: [k, P, 128, 2, HALF]; row (p, s) holds h_{2p+s} = out[2p+s]
    outs = np.stack([np.asarray(res.results[k]["op"]) for k in range(NCORES)])
    o = outs.astype(np.float32)                       # [k, p, 128, s, c]
    o = o.transpose(1, 3, 0, 2, 4)                    # [p, s, k, 128, c]
    o = o.reshape(T, NCORES, 2, H, HALF)              # [t, k, half, d, c]
    o = o.transpose(0, 4, 2, 1, 3)                    # [t, c, half, k, d]
    o = o.reshape(T, B, H)                            # j = 2c+half, b = 8j+k
    mask = (np.arange(T)[:, None] < lengths[None, :])
    o = np.where(mask[:, :, None], o, np.float32(0.0)).astype(np.float32)
    return o
